# revision 4
# baseline (speedup 1.0000x reference)
"""Trainium2 Bass kernel for the Connectivity_Branch GNN problem.

Pipeline (reference math):
  x[v] = relu(mean_agg[v]*wl + lb + X[v]@wr) + relu(maxdot[v] + lb1 + X[v]@wr1)
  h0 = relu(W0 @ x + b0); h1 = relu(W1 @ h0 + b1); out = relu(W2 @ h1 + b2)
  returns (x, out)

Sharding strategy (edge/graph parallel + tensor parallel, per hint):
  - Nodes are sorted by in-degree and assigned round-robin to the 8 cores so
    every core gets ~E/8 edges and an identical bucket structure (SPMD).
  - Each core aggregates its own nodes' edges (segment-sum for the mean path,
    per-feature segment-max for the max path) using degree-bucketed dense
    [128, T, 9, d] tiles and VectorE tensor_reduce.
  - W0 is column-sharded: each core computes a partial h0 = W0[:, mine] @ x_mine;
    one 4KB AllReduce combines partials. W1/W2 run redundantly on every core.
"""

import numpy as np
import ml_dtypes
from contextlib import ExitStack

import concourse.bass as bass
import concourse.bacc as bacc
import concourse.tile as tile
from concourse import mybir
from concourse.bass_utils import run_bass_kernel_spmd

N = 50000
E = 1600000
F = 8
HID = 1000
R = 100
NCORES = 8
P = 128
SEG = 512          # nodes per degree-bucket (per core), multiple of 128
W0CHUNK = 4        # k-tiles per W0 DMA chunk
FEAT_PAD = -128.0  # below any |w|*x value; exact in bf16

bf16 = ml_dtypes.bfloat16


def _plan_buckets(deg_ranked):
    """Bucket per-core node positions [0, N/8) into degree groups.

    Returns list of dicts with s/e (per-core position range), dh (padded
    degree), T (128-node tiles), off (column offset), foff (free-elem offset
    into the bf16 stream).
    """
    pos_total = N // NCORES
    buckets = []
    off = 0
    foff = 0
    s = 0
    while s < pos_total:
        e = min(s + SEG, pos_total)
        dh = int(max(1, deg_ranked[NCORES * s:NCORES * e].max()))
        T = (e - s + P - 1) // P
        buckets.append(dict(s=s, e=e, dh=dh, T=T, off=off, foff=foff))
        off += T
        foff += T * 9 * dh
        s = e
    return buckets, off, foff


def build_program(buckets, CT, SLEN, scalars):
    """scalars: dict with sign[F], wr[F], wr1[F], lb, lb1 floats (trace-time)."""
    dt = mybir.dt
    add = mybir.AluOpType.add
    mult = mybir.AluOpType.mult

    nc = bacc.Bacc("TRN2", target_bir_lowering=False, debug=False,
                   num_devices=NCORES)

    stream_d = nc.dram_tensor("stream", [P, SLEN], dt.bfloat16, kind="ExternalInput")
    xperm_d = nc.dram_tensor("xperm", [P, CT, F], dt.float32, kind="ExternalInput")
    invd_d = nc.dram_tensor("invd", [P, CT], dt.float32, kind="ExternalInput")
    mask_d = nc.dram_tensor("maskd", [P, CT], dt.float32, kind="ExternalInput")
    w0t_d = nc.dram_tensor("w0t", [CT, P, HID], dt.bfloat16, kind="ExternalInput")
    w1t_d = nc.dram_tensor("w1t", [8, P, HID], dt.float32, kind="ExternalInput")
    w2t_d = nc.dram_tensor("w2t", [8, P, R], dt.float32, kind="ExternalInput")
    b0_d = nc.dram_tensor("b0c", [P, 8], dt.float32, kind="ExternalInput")
    b1_d = nc.dram_tensor("b1c", [P, 8], dt.float32, kind="ExternalInput")
    b2_d = nc.dram_tensor("b2c", [R, 1], dt.float32, kind="ExternalInput")
    xout_d = nc.dram_tensor("xout", [P, CT], dt.float32, kind="ExternalOutput")
    res_d = nc.dram_tensor("res", [R, 1], dt.float32, kind="ExternalOutput")

    sign = scalars["sign"]
    wr = scalars["wr"]
    wr1 = scalars["wr1"]
    lb = scalars["lb"]
    lb1 = scalars["lb1"]

    with ExitStack() as ctx:
        tc = ctx.enter_context(tile.TileContext(nc))
        pers = ctx.enter_context(tc.tile_pool(name="pers", bufs=1))
        spool = ctx.enter_context(tc.tile_pool(name="spool", bufs=3))
        w0pool = ctx.enter_context(tc.tile_pool(name="w0pool", bufs=3))
        psum = ctx.enter_context(tc.tile_pool(name="psum", bufs=8, space="PSUM"))
        dram = ctx.enter_context(tc.tile_pool(name="dram", bufs=1, space="DRAM"))

        # ---- persistent SBUF state ----
        S_all = pers.tile([P, CT], dt.float32)
        M_all = pers.tile([P, CT, F], dt.float32)
        invd_sb = pers.tile([P, CT], dt.float32)
        mask_sb = pers.tile([P, CT], dt.float32)
        xperm_sb = pers.tile([P, CT, F], dt.float32)
        x_sb = pers.tile([P, CT], dt.float32)
        x_bf = pers.tile([P, CT], dt.bfloat16)
        xr = pers.tile([P, CT], dt.float32)
        xr1 = pers.tile([P, CT], dt.float32)
        md = pers.tile([P, CT], dt.float32)
        mean = pers.tile([P, CT], dt.float32)
        w1sb = pers.tile([P, 8, HID], dt.float32)
        w2sb = pers.tile([P, 8, R], dt.float32)
        b0sb = pers.tile([P, 8], dt.float32)
        b1sb = pers.tile([P, 8], dt.float32)
        b2sb = pers.tile([R, 1], dt.float32)
        h0p = pers.tile([P, 8], dt.float32)
        h0c = pers.tile([P, 8], dt.float32)
        h1c = pers.tile([P, 8], dt.float32)
        res_sb = pers.tile([R, 1], dt.float32)

        # ---- constant/small loads ----
        nc.sync.dma_start(out=invd_sb[:], in_=invd_d[:, :])
        nc.sync.dma_start(out=mask_sb[:], in_=mask_d[:, :])
        nc.sync.dma_start(out=xperm_sb[:], in_=xperm_d[:, :, :])
        nc.sync.dma_start(out=w1sb[:], in_=w1t_d.ap().rearrange("k p h -> p k h"))
        nc.sync.dma_start(out=w2sb[:], in_=w2t_d.ap().rearrange("k p r -> p k r"))
        nc.sync.dma_start(out=b0sb[:], in_=b0_d[:, :])
        nc.sync.dma_start(out=b1sb[:], in_=b1_d[:, :])
        nc.sync.dma_start(out=b2sb[:], in_=b2_d[:, :])

        # ---- Phase A: per-bucket segment reductions ----
        for b in buckets:
            T, dh, off, foff = b["T"], b["dh"], b["off"], b["foff"]
            L = T * 9 * dh
            st = spool.tile([P, T, 9, dh], dt.bfloat16, tag="stream")
            nc.sync.dma_start(
                out=st[:],
                in_=stream_d[:, foff:foff + L].rearrange(
                    "p (t n d) -> p t n d", t=T, n=9),
            )
            nc.vector.tensor_reduce(
                out=S_all[:, off:off + T],
                in_=st[:, :, 0, :],
                axis=mybir.AxisListType.X,
                op=add,
            )
            nc.vector.tensor_reduce(
                out=M_all[:, off:off + T, :],
                in_=st[:, :, 1:9, :],
                axis=mybir.AxisListType.X,
                op=mybir.AluOpType.max,
            )

        # ---- Phase A2: combine to x ----
        # xr = X @ wr ; xr1 = X @ wr1 ; md = sum_f sign_f * M_f
        nc.vector.tensor_scalar_mul(xr[:], xperm_sb[:, :, 0], float(wr[0]))
        nc.vector.tensor_scalar_mul(xr1[:], xperm_sb[:, :, 0], float(wr1[0]))
        nc.vector.tensor_scalar_mul(md[:], M_all[:, :, 0], float(sign[0]))
        for f in range(1, F):
            nc.vector.scalar_tensor_tensor(
                out=xr[:], in0=xperm_sb[:, :, f], scalar=float(wr[f]),
                in1=xr[:], op0=mult, op1=add)
            nc.vector.scalar_tensor_tensor(
                out=xr1[:], in0=xperm_sb[:, :, f], scalar=float(wr1[f]),
                in1=xr1[:], op0=mult, op1=add)
            nc.vector.scalar_tensor_tensor(
                out=md[:], in0=M_all[:, :, f], scalar=float(sign[f]),
                in1=md[:], op0=mult, op1=add)
        # mean = S * invd ; pre1 = (xr + lb) + mean ; relu
        nc.vector.tensor_tensor(out=mean[:], in0=S_all[:], in1=invd_sb[:], op=mult)
        nc.vector.scalar_tensor_tensor(
            out=mean[:], in0=xr[:], scalar=float(lb), in1=mean[:], op0=add, op1=add)
        nc.vector.tensor_scalar_max(mean[:], mean[:], 0.0)
        # md = md * mask ; pre2 = (xr1 + lb1) + md ; relu
        nc.vector.tensor_tensor(out=md[:], in0=md[:], in1=mask_sb[:], op=mult)
        nc.vector.scalar_tensor_tensor(
            out=md[:], in0=xr1[:], scalar=float(lb1), in1=md[:], op0=add, op1=add)
        nc.vector.tensor_scalar_max(md[:], md[:], 0.0)
        nc.vector.tensor_tensor(out=x_sb[:], in0=mean[:], in1=md[:], op=add)
        nc.vector.tensor_copy(out=x_bf[:], in_=x_sb[:])
        nc.sync.dma_start(out=xout_d[:, :], in_=x_sb[:])

        # ---- Phase B: partial h0 = W0[:, mine] @ x_mine ----
        h0psum = [psum.tile([P, 1], dt.float32, tag="mv", name=f"h0psum{j}")
                  for j in range(8)]
        jsl = [(jt * P, min(HID, (jt + 1) * P)) for jt in range(8)]
        nchunks = (CT + W0CHUNK - 1) // W0CHUNK
        for c in range(nchunks):
            t0 = c * W0CHUNK
            ck = min(W0CHUNK, CT - t0)
            w0c = w0pool.tile([P, ck, HID], dt.bfloat16, tag="w0c")
            nc.sync.dma_start(
                out=w0c[:],
                in_=w0t_d[t0:t0 + ck, :, :].rearrange("t p h -> p t h"),
            )
            for i in range(ck):
                t = t0 + i
                for jt, (js, je) in enumerate(jsl):
                    nc.tensor.matmul(
                        h0psum[jt][0:je - js, 0:1],
                        lhsT=w0c[:, i, js:je],
                        rhs=x_bf[:, t:t + 1],
                        start=(t == 0),
                        stop=(t == CT - 1),
                    )

        # ---- Phase C: AllReduce partial h0, then W1/W2 ----
        nc.vector.memset(h0p[:], 0.0)
        for jt, (js, je) in enumerate(jsl):
            nc.vector.tensor_copy(out=h0p[0:je - js, jt:jt + 1],
                                  in_=h0psum[jt][0:je - js, 0:1])
        arin = dram.tile([P, 8], dt.float32)
        arout = dram.tile([P, 8], dt.float32)
        nc.sync.dma_start(out=arin[:], in_=h0p[:])
        nc.gpsimd.collective_compute(
            "AllReduce",
            add,
            replica_groups=[list(range(NCORES))],
            ins=[arin[:].opt()],
            outs=[arout[:].opt()],
        )
        nc.sync.dma_start(out=h0c[:], in_=arout[:])
        # h0 = relu(h0 + b0)
        nc.vector.tensor_tensor(out=h0c[:], in0=h0c[:], in1=b0sb[:], op=add)
        nc.vector.tensor_scalar_max(h0c[:], h0c[:], 0.0)

        h1psum = [psum.tile([P, 1], dt.float32, tag="mv", name=f"h1psum{j}")
                  for j in range(8)]
        for kt in range(8):
            for jt, (js, je) in enumerate(jsl):
                nc.tensor.matmul(
                    h1psum[jt][0:je - js, 0:1],
                    lhsT=w1sb[:, kt, js:je],
                    rhs=h0c[:, kt:kt + 1],
                    start=(kt == 0),
                    stop=(kt == 7),
                )
        nc.vector.memset(h1c[:], 0.0)
        for jt, (js, je) in enumerate(jsl):
            nc.vector.tensor_copy(out=h1c[0:je - js, jt:jt + 1],
                                  in_=h1psum[jt][0:je - js, 0:1])
        nc.vector.tensor_tensor(out=h1c[:], in0=h1c[:], in1=b1sb[:], op=add)
        nc.vector.tensor_scalar_max(h1c[:], h1c[:], 0.0)

        res_psum = psum.tile([P, 1], dt.float32, tag="mv")
        for kt in range(8):
            nc.tensor.matmul(
                res_psum[0:R, 0:1],
                lhsT=w2sb[:, kt, :],
                rhs=h1c[:, kt:kt + 1],
                start=(kt == 0),
                stop=(kt == 7),
            )
        nc.vector.tensor_copy(out=res_sb[:], in_=res_psum[0:R, 0:1])
        nc.vector.tensor_tensor(out=res_sb[:], in0=res_sb[:], in1=b2sb[:], op=add)
        nc.vector.tensor_scalar_max(res_sb[:], res_sb[:], 0.0)
        nc.sync.dma_start(out=res_d[:, :], in_=res_sb[:])

    nc.compile()
    return nc


def prepare(X, edge_index, lin_l_w, lin_l_b, lin_r_w,
            lin_l1_w, lin_l1_b, lin_r1_w, W0, b0, W1, b1, W2, b2):
    """Host-side sharding/layout. Returns (nc-inputs, bookkeeping)."""
    X = np.asarray(X, np.float32)
    src = np.asarray(edge_index[0], np.int64)
    dst = np.asarray(edge_index[1], np.int64)
    wl = np.asarray(lin_l_w, np.float32).reshape(F)
    lb = float(np.asarray(lin_l_b).reshape(()))
    wr = np.asarray(lin_r_w, np.float32).reshape(F)
    wl1 = np.asarray(lin_l1_w, np.float32).reshape(F)
    lb1 = float(np.asarray(lin_l1_b).reshape(()))
    wr1 = np.asarray(lin_r1_w, np.float32).reshape(F)
    W0 = np.asarray(W0, np.float32)
    b0 = np.asarray(b0, np.float32)
    W1 = np.asarray(W1, np.float32)
    b1 = np.asarray(b1, np.float32)
    W2 = np.asarray(W2, np.float32)
    b2 = np.asarray(b2, np.float32)

    deg = np.bincount(dst, minlength=N)
    order = np.argsort(deg, kind="stable")      # rank -> old id (deg ascending)
    rank = np.empty(N, np.int64)
    rank[order] = np.arange(N)
    deg_ranked = deg[order]

    buckets, CT, SLEN = _plan_buckets(deg_ranked)

    # edges grouped by rank of dst
    edge_order = np.argsort(rank[dst], kind="stable")
    src_sorted = src[edge_order]
    starts = np.zeros(N + 1, np.int64)
    np.cumsum(deg_ranked, out=starts[1:])

    # per-node payload tables
    Y = X @ wl                                   # [N]
    V8 = np.abs(wl1)[None, :] * X                # [N, F]
    sign = np.where(wl1 >= 0.0, 1.0, -1.0).astype(np.float32)

    in_maps = []
    owners = []
    for c in range(NCORES):
        stream = np.zeros((P, SLEN), dtype=bf16)
        owner = np.full((P, CT), -1, np.int64)
        invd = np.ones((P, CT), np.float32)
        maskd = np.zeros((P, CT), np.float32)
        for b in buckets:
            s, e, dh, T, off, foff = b["s"], b["e"], b["dh"], b["T"], b["off"], b["foff"]
            n = e - s
            r = NCORES * np.arange(s, e) + c     # global ranks of these nodes
            o = order[r]
            dd = deg_ranked[r]
            idx = starts[r][:, None] + np.arange(dh)[None, :]
            valid = np.arange(dh)[None, :] < dd[:, None]
            sg = src_sorted[np.clip(idx, 0, E - 1)]
            yv = np.where(valid, Y[sg], 0.0).astype(np.float32)       # [n, dh]
            fv = np.where(valid[:, :, None], V8[sg], FEAT_PAD)        # [n, dh, F]
            payload = np.empty((n, 9, dh), np.float32)
            payload[:, 0, :] = yv
            payload[:, 1:, :] = fv.transpose(0, 2, 1)
            # slot i -> (t = i//P, p = i%P)
            block = np.full((T * P, 9, dh), 0.0, np.float32)
            block[:, 1:, :] = FEAT_PAD
            block[:n] = payload
            block = block.reshape(T, P, 9, dh).transpose(1, 0, 2, 3)  # [P,T,9,dh]
            stream[:, foff:foff + T * 9 * dh] = \
                block.reshape(P, T * 9 * dh).astype(bf16)
            iarr = np.arange(n)
            pp, tt = iarr % P, iarr // P
            owner[pp, off + tt] = o
            invd[pp, off + tt] = 1.0 / np.maximum(dd, 1)
            maskd[pp, off + tt] = (dd > 0).astype(np.float32)
        owners.append(owner)
        xperm = np.zeros((P, CT, F), np.float32)
        ovalid = owner >= 0
        xperm[ovalid] = X[owner[ovalid]]
        in_maps.append(dict(stream=stream, xperm=xperm, invd=invd, maskd=maskd,
                            owner=owner))

    # W0 column shards (bf16), packed [CT, P, HID]
    W0T_bf = np.ascontiguousarray(W0.T).astype(bf16)    # [N, HID]
    zrow = np.zeros((1, HID), dtype=bf16)
    W0T_bfz = np.concatenate([W0T_bf, zrow], axis=0)    # id -1 -> zero row
    for c in range(NCORES):
        ow = in_maps[c]["owner"].T                       # [CT, P]
        in_maps[c]["w0t"] = np.ascontiguousarray(W0T_bfz[ow])  # [CT, P, HID]

    # W1/W2 prepack (same on all cores)
    W1T = np.zeros((8 * P, HID), np.float32)
    W1T[:HID] = W1.T
    w1t = W1T.reshape(8, P, HID)
    W2T = np.zeros((8 * P, R), np.float32)
    W2T[:HID] = W2.T
    w2t = W2T.reshape(8, P, R)
    bp = np.zeros(8 * P, np.float32)
    bp[:HID] = b0
    b0c = bp.reshape(8, P).T.copy()
    bp = np.zeros(8 * P, np.float32)
    bp[:HID] = b1
    b1c = bp.reshape(8, P).T.copy()
    b2c = b2.reshape(R, 1).copy()
    for c in range(NCORES):
        m = in_maps[c]
        m.pop("owner")
        m["w1t"] = w1t
        m["w2t"] = w2t
        m["b0c"] = b0c
        m["b1c"] = b1c
        m["b2c"] = b2c

    scalars = dict(sign=sign, wr=wr, wr1=wr1, lb=lb, lb1=lb1)
    book = dict(buckets=buckets, CT=CT, SLEN=SLEN, owners=owners,
                scalars=scalars)
    return in_maps, book


_RESULT_CACHE = {}


def kernel(**inputs):
    in_maps, book = prepare(**inputs)
    nc = build_program(book["buckets"], book["CT"], book["SLEN"],
                       book["scalars"])
    res = run_bass_kernel_spmd(nc, in_maps, core_ids=list(range(NCORES)),
                               trace=False)
    _RESULT_CACHE["last"] = res
    X_emb = np.zeros(N, np.float32)
    for c in range(NCORES):
        owner = book["owners"][c]
        ov = owner >= 0
        X_emb[owner[ov]] = res.results[c]["xout"][ov]
    out = res.results[0]["res"].reshape(R).astype(np.float32)
    return (X_emb, out)


# revision 5
# speedup vs baseline: 1.2505x; 1.2505x over previous
"""Trainium2 Bass kernel for the Connectivity_Branch GNN problem.

Pipeline (reference math):
  x[v] = relu(mean_agg[v]*wl + lb + X[v]@wr) + relu(maxdot[v] + lb1 + X[v]@wr1)
  h0 = relu(W0 @ x + b0); h1 = relu(W1 @ h0 + b1); out = relu(W2 @ h1 + b2)
  returns (x, out)

Sharding strategy (edge/graph parallel + tensor parallel, per hint):
  - Nodes are sorted by in-degree and assigned round-robin to the 8 cores so
    every core gets ~E/8 edges and an identical bucket structure (SPMD).
  - Each core aggregates its own nodes' edges (segment-sum for the mean path,
    per-feature segment-max for the max path) using degree-bucketed dense
    [128, T, 9, d] tiles and VectorE tensor_reduce.
  - W0 is column-sharded: each core computes a partial h0 = W0[:, mine] @ x_mine;
    one 4KB AllReduce combines partials. W1/W2 run redundantly on every core.
"""

import numpy as np
import ml_dtypes
from contextlib import ExitStack

import concourse.bass as bass
import concourse.bacc as bacc
import concourse.tile as tile
from concourse import mybir
from concourse.bass_utils import run_bass_kernel_spmd

N = 50000
E = 1600000
F = 8
HID = 1000
R = 100
NCORES = 8
P = 128
SEG = 512          # nodes per degree-bucket (per core), multiple of 128
W0CHUNK = 7        # k-tiles per W0 DMA chunk
FEAT_PAD = -128.0  # below any |w|*x value; exact in bf16

bf16 = ml_dtypes.bfloat16


def _plan_buckets(deg_ranked):
    """Bucket per-core node positions [0, N/8) into degree groups.

    Returns list of dicts with s/e (per-core position range), dh (padded
    degree), T (128-node tiles), off (column offset), foff (free-elem offset
    into the bf16 stream).
    """
    pos_total = N // NCORES
    buckets = []
    off = 0
    foff = 0
    s = 0
    while s < pos_total:
        e = min(s + SEG, pos_total)
        dh = int(max(1, deg_ranked[NCORES * s:NCORES * e].max()))
        T = (e - s + P - 1) // P
        buckets.append(dict(s=s, e=e, dh=dh, T=T, off=off, foff=foff))
        off += T
        foff += T * 9 * dh
        s = e
    return buckets, off, foff


def build_program(buckets, CT, SLEN, scalars):
    """scalars: dict with sign[F], wr[F], wr1[F], lb, lb1 floats (trace-time)."""
    dt = mybir.dt
    add = mybir.AluOpType.add
    mult = mybir.AluOpType.mult

    nc = bacc.Bacc("TRN2", target_bir_lowering=False, debug=False,
                   num_devices=NCORES)

    stream_d = nc.dram_tensor("stream", [P, SLEN], dt.bfloat16, kind="ExternalInput")
    xperm_d = nc.dram_tensor("xperm", [P, CT, F], dt.float32, kind="ExternalInput")
    invd_d = nc.dram_tensor("invd", [P, CT], dt.float32, kind="ExternalInput")
    mask_d = nc.dram_tensor("maskd", [P, CT], dt.float32, kind="ExternalInput")
    w0t_d = nc.dram_tensor("w0t", [CT, P, HID], dt.bfloat16, kind="ExternalInput")
    w1t_d = nc.dram_tensor("w1t", [8, P, HID], dt.bfloat16, kind="ExternalInput")
    w2t_d = nc.dram_tensor("w2t", [8, P, R], dt.bfloat16, kind="ExternalInput")
    b0_d = nc.dram_tensor("b0c", [P, 8], dt.float32, kind="ExternalInput")
    b1_d = nc.dram_tensor("b1c", [P, 8], dt.float32, kind="ExternalInput")
    b2_d = nc.dram_tensor("b2c", [R, 1], dt.float32, kind="ExternalInput")
    xout_d = nc.dram_tensor("xout", [P, CT], dt.float32, kind="ExternalOutput")
    res_d = nc.dram_tensor("res", [R, 1], dt.float32, kind="ExternalOutput")

    sign = scalars["sign"]
    wr = scalars["wr"]
    wr1 = scalars["wr1"]
    lb = scalars["lb"]
    lb1 = scalars["lb1"]

    with ExitStack() as ctx:
        tc = ctx.enter_context(tile.TileContext(nc))
        pers = ctx.enter_context(tc.tile_pool(name="pers", bufs=1))
        spool = ctx.enter_context(tc.tile_pool(name="spool", bufs=3))
        w0pool = ctx.enter_context(tc.tile_pool(name="w0pool", bufs=4))
        psum = ctx.enter_context(tc.tile_pool(name="psum", bufs=8, space="PSUM"))
        dram = ctx.enter_context(tc.tile_pool(name="dram", bufs=1, space="DRAM"))

        # ---- persistent SBUF state ----
        S_all = pers.tile([P, CT], dt.float32)
        M_all = pers.tile([P, CT, F], dt.float32)
        invd_sb = pers.tile([P, CT], dt.float32)
        mask_sb = pers.tile([P, CT], dt.float32)
        xperm_sb = pers.tile([P, CT, F], dt.float32)
        x_sb = pers.tile([P, CT], dt.float32)
        x_bf = pers.tile([P, CT], dt.bfloat16)
        xr = pers.tile([P, CT], dt.float32)
        xr1 = pers.tile([P, CT], dt.float32)
        md = pers.tile([P, CT], dt.float32)
        mean = pers.tile([P, CT], dt.float32)
        w1sb = pers.tile([P, 8, HID], dt.bfloat16)
        w2sb = pers.tile([P, 8, R], dt.bfloat16)
        b0sb = pers.tile([P, 8], dt.float32)
        b1sb = pers.tile([P, 8], dt.float32)
        b2sb = pers.tile([R, 1], dt.float32)
        h0p = pers.tile([P, 8], dt.float32)
        h0c = pers.tile([P, 8], dt.float32)
        h0cb = pers.tile([P, 8], dt.bfloat16)
        h1c = pers.tile([P, 8], dt.float32)
        h1cb = pers.tile([P, 8], dt.bfloat16)
        res_sb = pers.tile([R, 1], dt.float32)

        # ---- constant/small loads ----
        nc.sync.dma_start(out=invd_sb[:], in_=invd_d[:, :])
        nc.sync.dma_start(out=mask_sb[:], in_=mask_d[:, :])
        nc.sync.dma_start(out=xperm_sb[:], in_=xperm_d[:, :, :])
        nc.sync.dma_start(out=w1sb[:], in_=w1t_d.ap().rearrange("k p h -> p k h"))
        nc.sync.dma_start(out=w2sb[:], in_=w2t_d.ap().rearrange("k p r -> p k r"))
        nc.sync.dma_start(out=b0sb[:], in_=b0_d[:, :])
        nc.sync.dma_start(out=b1sb[:], in_=b1_d[:, :])
        nc.sync.dma_start(out=b2sb[:], in_=b2_d[:, :])

        # ---- Phase A: per-bucket segment reductions ----
        for b in buckets:
            T, dh, off, foff = b["T"], b["dh"], b["off"], b["foff"]
            L = T * 9 * dh
            st = spool.tile([P, T, 9, dh], dt.bfloat16, tag="stream")
            nc.sync.dma_start(
                out=st[:],
                in_=stream_d[:, foff:foff + L].rearrange(
                    "p (t n d) -> p t n d", t=T, n=9),
            )
            nc.vector.tensor_reduce(
                out=S_all[:, off:off + T],
                in_=st[:, :, 0, :],
                axis=mybir.AxisListType.X,
                op=add,
            )
            nc.vector.tensor_reduce(
                out=M_all[:, off:off + T, :],
                in_=st[:, :, 1:9, :],
                axis=mybir.AxisListType.X,
                op=mybir.AluOpType.max,
            )

        # ---- Phase A2: combine to x ----
        # xr = X @ wr ; xr1 = X @ wr1 ; md = sum_f sign_f * M_f
        nc.vector.tensor_scalar_mul(xr[:], xperm_sb[:, :, 0], float(wr[0]))
        nc.vector.tensor_scalar_mul(xr1[:], xperm_sb[:, :, 0], float(wr1[0]))
        nc.vector.tensor_scalar_mul(md[:], M_all[:, :, 0], float(sign[0]))
        for f in range(1, F):
            nc.vector.scalar_tensor_tensor(
                out=xr[:], in0=xperm_sb[:, :, f], scalar=float(wr[f]),
                in1=xr[:], op0=mult, op1=add)
            nc.vector.scalar_tensor_tensor(
                out=xr1[:], in0=xperm_sb[:, :, f], scalar=float(wr1[f]),
                in1=xr1[:], op0=mult, op1=add)
            nc.vector.scalar_tensor_tensor(
                out=md[:], in0=M_all[:, :, f], scalar=float(sign[f]),
                in1=md[:], op0=mult, op1=add)
        # mean = S * invd ; pre1 = (xr + lb) + mean ; relu
        nc.vector.tensor_tensor(out=mean[:], in0=S_all[:], in1=invd_sb[:], op=mult)
        nc.vector.scalar_tensor_tensor(
            out=mean[:], in0=xr[:], scalar=float(lb), in1=mean[:], op0=add, op1=add)
        nc.vector.tensor_scalar_max(mean[:], mean[:], 0.0)
        # md = md * mask ; pre2 = (xr1 + lb1) + md ; relu
        nc.vector.tensor_tensor(out=md[:], in0=md[:], in1=mask_sb[:], op=mult)
        nc.vector.scalar_tensor_tensor(
            out=md[:], in0=xr1[:], scalar=float(lb1), in1=md[:], op0=add, op1=add)
        nc.vector.tensor_scalar_max(md[:], md[:], 0.0)
        nc.vector.tensor_tensor(out=x_sb[:], in0=mean[:], in1=md[:], op=add)
        nc.vector.tensor_copy(out=x_bf[:], in_=x_sb[:])
        nc.sync.dma_start(out=xout_d[:, :], in_=x_sb[:])

        # ---- Phase B: partial h0 = W0[:, mine] @ x_mine ----
        h0psum = [psum.tile([P, 1], dt.float32, tag="mv", name=f"h0psum{j}")
                  for j in range(8)]
        jsl = [(jt * P, min(HID, (jt + 1) * P)) for jt in range(8)]
        nchunks = (CT + W0CHUNK - 1) // W0CHUNK
        for c in range(nchunks):
            t0 = c * W0CHUNK
            ck = min(W0CHUNK, CT - t0)
            w0c = w0pool.tile([P, ck, HID], dt.bfloat16, tag="w0c")
            nc.sync.dma_start(
                out=w0c[:],
                in_=w0t_d[t0:t0 + ck, :, :].rearrange("t p h -> p t h"),
            )
            for i in range(ck):
                t = t0 + i
                for jt, (js, je) in enumerate(jsl):
                    nc.tensor.matmul(
                        h0psum[jt][0:je - js, 0:1],
                        lhsT=w0c[:, i, js:je],
                        rhs=x_bf[:, t:t + 1],
                        start=(t == 0),
                        stop=(t == CT - 1),
                    )

        # ---- Phase C: AllReduce partial h0, then W1/W2 ----
        nc.vector.memset(h0p[:], 0.0)
        for jt, (js, je) in enumerate(jsl):
            nc.vector.tensor_copy(out=h0p[0:je - js, jt:jt + 1],
                                  in_=h0psum[jt][0:je - js, 0:1])
        arin = dram.tile([P, 8], dt.float32)
        arout = dram.tile([NCORES * P, 8], dt.float32)
        nc.sync.dma_start(out=arin[:], in_=h0p[:])
        nc.gpsimd.collective_compute(
            "AllGather",
            mybir.AluOpType.bypass,
            replica_groups=[list(range(NCORES))],
            ins=[arin[:].opt()],
            outs=[arout[:].opt()],
        )
        h0g = pers.tile([P, NCORES, 8], dt.float32)
        nc.sync.dma_start(
            out=h0g[:],
            in_=arout[:, :].rearrange("(c p) j -> p c j", c=NCORES))
        # reduce over the core axis (innermost via strided view), add bias, relu
        nc.vector.tensor_reduce(
            out=h0c[:],
            in_=h0g[:].rearrange("p c j -> p j c"),
            axis=mybir.AxisListType.X,
            op=add,
        )
        nc.vector.tensor_tensor(out=h0c[:], in0=h0c[:], in1=b0sb[:], op=add)
        nc.vector.tensor_scalar_max(h0c[:], h0c[:], 0.0)
        nc.vector.tensor_copy(out=h0cb[:], in_=h0c[:])

        h1psum = [psum.tile([P, 1], dt.float32, tag="mv", name=f"h1psum{j}")
                  for j in range(8)]
        for kt in range(8):
            for jt, (js, je) in enumerate(jsl):
                nc.tensor.matmul(
                    h1psum[jt][0:je - js, 0:1],
                    lhsT=w1sb[:, kt, js:je],
                    rhs=h0cb[:, kt:kt + 1],
                    start=(kt == 0),
                    stop=(kt == 7),
                )
        nc.vector.memset(h1c[:], 0.0)
        for jt, (js, je) in enumerate(jsl):
            nc.vector.tensor_copy(out=h1c[0:je - js, jt:jt + 1],
                                  in_=h1psum[jt][0:je - js, 0:1])
        nc.vector.tensor_tensor(out=h1c[:], in0=h1c[:], in1=b1sb[:], op=add)
        nc.vector.tensor_scalar_max(h1c[:], h1c[:], 0.0)
        nc.vector.tensor_copy(out=h1cb[:], in_=h1c[:])

        res_psum = psum.tile([P, 1], dt.float32, tag="mv")
        for kt in range(8):
            nc.tensor.matmul(
                res_psum[0:R, 0:1],
                lhsT=w2sb[:, kt, :],
                rhs=h1cb[:, kt:kt + 1],
                start=(kt == 0),
                stop=(kt == 7),
            )
        nc.vector.tensor_copy(out=res_sb[:], in_=res_psum[0:R, 0:1])
        nc.vector.tensor_tensor(out=res_sb[:], in0=res_sb[:], in1=b2sb[:], op=add)
        nc.vector.tensor_scalar_max(res_sb[:], res_sb[:], 0.0)
        nc.sync.dma_start(out=res_d[:, :], in_=res_sb[:])

    nc.compile()
    return nc


def prepare(X, edge_index, lin_l_w, lin_l_b, lin_r_w,
            lin_l1_w, lin_l1_b, lin_r1_w, W0, b0, W1, b1, W2, b2):
    """Host-side sharding/layout. Returns (nc-inputs, bookkeeping)."""
    X = np.asarray(X, np.float32)
    src = np.asarray(edge_index[0], np.int64)
    dst = np.asarray(edge_index[1], np.int64)
    wl = np.asarray(lin_l_w, np.float32).reshape(F)
    lb = float(np.asarray(lin_l_b).reshape(()))
    wr = np.asarray(lin_r_w, np.float32).reshape(F)
    wl1 = np.asarray(lin_l1_w, np.float32).reshape(F)
    lb1 = float(np.asarray(lin_l1_b).reshape(()))
    wr1 = np.asarray(lin_r1_w, np.float32).reshape(F)
    W0 = np.asarray(W0, np.float32)
    b0 = np.asarray(b0, np.float32)
    W1 = np.asarray(W1, np.float32)
    b1 = np.asarray(b1, np.float32)
    W2 = np.asarray(W2, np.float32)
    b2 = np.asarray(b2, np.float32)

    deg = np.bincount(dst, minlength=N)
    order = np.argsort(deg, kind="stable")      # rank -> old id (deg ascending)
    rank = np.empty(N, np.int64)
    rank[order] = np.arange(N)
    deg_ranked = deg[order]

    buckets, CT, SLEN = _plan_buckets(deg_ranked)

    # edges grouped by rank of dst
    edge_order = np.argsort(rank[dst], kind="stable")
    src_sorted = src[edge_order]
    starts = np.zeros(N + 1, np.int64)
    np.cumsum(deg_ranked, out=starts[1:])

    # per-node payload tables
    Y = X @ wl                                   # [N]
    V8 = np.abs(wl1)[None, :] * X                # [N, F]
    sign = np.where(wl1 >= 0.0, 1.0, -1.0).astype(np.float32)

    in_maps = []
    owners = []
    for c in range(NCORES):
        stream = np.zeros((P, SLEN), dtype=bf16)
        owner = np.full((P, CT), -1, np.int64)
        invd = np.ones((P, CT), np.float32)
        maskd = np.zeros((P, CT), np.float32)
        for b in buckets:
            s, e, dh, T, off, foff = b["s"], b["e"], b["dh"], b["T"], b["off"], b["foff"]
            n = e - s
            r = NCORES * np.arange(s, e) + c     # global ranks of these nodes
            o = order[r]
            dd = deg_ranked[r]
            idx = starts[r][:, None] + np.arange(dh)[None, :]
            valid = np.arange(dh)[None, :] < dd[:, None]
            sg = src_sorted[np.clip(idx, 0, E - 1)]
            yv = np.where(valid, Y[sg], 0.0).astype(np.float32)       # [n, dh]
            fv = np.where(valid[:, :, None], V8[sg], FEAT_PAD)        # [n, dh, F]
            payload = np.empty((n, 9, dh), np.float32)
            payload[:, 0, :] = yv
            payload[:, 1:, :] = fv.transpose(0, 2, 1)
            # slot i -> (t = i//P, p = i%P)
            block = np.full((T * P, 9, dh), 0.0, np.float32)
            block[:, 1:, :] = FEAT_PAD
            block[:n] = payload
            block = block.reshape(T, P, 9, dh).transpose(1, 0, 2, 3)  # [P,T,9,dh]
            stream[:, foff:foff + T * 9 * dh] = \
                block.reshape(P, T * 9 * dh).astype(bf16)
            iarr = np.arange(n)
            pp, tt = iarr % P, iarr // P
            owner[pp, off + tt] = o
            invd[pp, off + tt] = 1.0 / np.maximum(dd, 1)
            maskd[pp, off + tt] = (dd > 0).astype(np.float32)
        owners.append(owner)
        xperm = np.zeros((P, CT, F), np.float32)
        ovalid = owner >= 0
        xperm[ovalid] = X[owner[ovalid]]
        in_maps.append(dict(stream=stream, xperm=xperm, invd=invd, maskd=maskd,
                            owner=owner))

    # W0 column shards (bf16), packed [CT, P, HID]
    W0T_bf = np.ascontiguousarray(W0.T).astype(bf16)    # [N, HID]
    zrow = np.zeros((1, HID), dtype=bf16)
    W0T_bfz = np.concatenate([W0T_bf, zrow], axis=0)    # id -1 -> zero row
    for c in range(NCORES):
        ow = in_maps[c]["owner"].T                       # [CT, P]
        in_maps[c]["w0t"] = np.ascontiguousarray(W0T_bfz[ow])  # [CT, P, HID]

    # W1/W2 prepack (same on all cores)
    W1T = np.zeros((8 * P, HID), np.float32)
    W1T[:HID] = W1.T
    w1t = W1T.reshape(8, P, HID).astype(bf16)
    W2T = np.zeros((8 * P, R), np.float32)
    W2T[:HID] = W2.T
    w2t = W2T.reshape(8, P, R).astype(bf16)
    bp = np.zeros(8 * P, np.float32)
    bp[:HID] = b0
    b0c = bp.reshape(8, P).T.copy()
    bp = np.zeros(8 * P, np.float32)
    bp[:HID] = b1
    b1c = bp.reshape(8, P).T.copy()
    b2c = b2.reshape(R, 1).copy()
    for c in range(NCORES):
        m = in_maps[c]
        m.pop("owner")
        m["w1t"] = w1t
        m["w2t"] = w2t
        m["b0c"] = b0c
        m["b1c"] = b1c
        m["b2c"] = b2c

    scalars = dict(sign=sign, wr=wr, wr1=wr1, lb=lb, lb1=lb1)
    book = dict(buckets=buckets, CT=CT, SLEN=SLEN, owners=owners,
                scalars=scalars)
    return in_maps, book


_RESULT_CACHE = {}


def kernel(**inputs):
    in_maps, book = prepare(**inputs)
    nc = build_program(book["buckets"], book["CT"], book["SLEN"],
                       book["scalars"])
    res = run_bass_kernel_spmd(nc, in_maps, core_ids=list(range(NCORES)),
                               trace=False)
    _RESULT_CACHE["last"] = res
    X_emb = np.zeros(N, np.float32)
    for c in range(NCORES):
        owner = book["owners"][c]
        ov = owner >= 0
        X_emb[owner[ov]] = res.results[c]["xout"][ov]
    out = res.results[0]["res"].reshape(R).astype(np.float32)
    return (X_emb, out)
